# revision 64
# baseline (speedup 1.0000x reference)
"""GAT layer (4096 nodes, 8 heads, 64 feat/head) on 8 Trainium2 NeuronCores.

Sharding: node dim i (rows of x/adj/output) split 8 ways; W/a replicated;
each core computes attention+aggregation for its 512-row block against all
4096 j-nodes (Wh computed redundantly per core from the full x).

Math (per core, per head h):
  s_i = e_src[i,h], d_j = e_dst[j,h]
  exp(lrelu(s+d)) = exp(0.2 s_i) * max(q_j, E8_i * expd_j)
  with q_j = exp(0.2 d_j), E8_i = exp(0.8 s_i), expd_j = exp(d_j);
  the factor exp(0.2 s_i) cancels in the softmax ratio, so
  g[j,i] = adj[i,j] * max(q_j, E8_i * expd_j) is the unnormalized attention,
  out[i,d] = relu( (sum_j g[j,i] Wh[j,d]) / (sum_j g[j,i]) ),
  both sums from ONE matmul per (j-chunk, head) with rhs=g, lhsT=[Wh_h|ones].
  The tiny attention projections are folded on the host alongside W @ a:
  exp(0.8 e_src) rows (broadcast on-device by a K=8 selector matmul) and
  the per-j scalars exp(0.2 e_dst)/exp(e_dst) (DMA'd in tile layout) --
  ~17 MFLOP of host numpy that deletes every device exponential, the Exp
  table load, and the e-vector matmul/ACT chain from the critical path.

Per [128j x 512i] tile the attention needs only TWO element-wise steps:
  (1) m = max(E8 * expd_scalar, q_scalar): dual-op DVE tensor_scalar on
      all-bf16 operands (4x DVE mode, ~194ns); no per-tile ACT exp at all
      (the factored form needs only multiplies, and scalar operands are
      exempt from the 2-byte rule so evq/evd stay fp32).
  (2) g = m * adj: ONE quad-batched tensor_tensor over the pass's 4 heads
      ([128, 4*512] bf16, broadcast mask): DVE 2x mode (~1.1us) for 42
      passes, GPSIMD for 22 (computed balance point; gpsimd tensor_tensor
      only supports add/mult on real HW -- min/max fail the ISA check).

Schedule (all within one SBUF pool scope -- separate scopes would alias
phase-1 tiles and serialize):
  - Wh staging (x@W matmuls + ACT copies into bf16 [Wh_h|1] lhsT blocks)
    is software-pipelined INTO the head-group-0 pass loop (lookahead 4),
    so passes start at ~6us, and the dense PE stream holds the Tensor
    engine at full p-state (cost model: idle PE decays to a 2-3.7x-slower
    p-state; 4 identity-transpose warm-ups cover the DMA window).
  - tensor_scalar ops are emitted two passes ahead of the mask so the
    GPSIMD mask passes never wait behind DVE's own masks; 8-deep g4 ring.
  - finalize: PE transposes [65,512]->[i,e], DVE reciprocal, then the
    Relu-and-scale split ACT/DVE per head (activation scale accepts a
    per-partition AP), so the tail runs on three engines in parallel;
    head-group-0 finalize is interleaved into sweep 2's passes; the
    final pass is split into head pairs so heads 4-5 finalize while 6-7
    still accumulate; the last 3 passes never mask on GPSIMD so its
    4.2us op stays off the exit path.
Per-core engine busy: DVE 99.3us (87% of the 113.9us span, gapless
mid-stream), GPSIMD 92.5us, PE 72.4us, ACT 41us. bf16 g/Wh keep ~3e-3
max rel error (gate 2e-2); GAT_EXACT=1 switches to f32/f32r (slower).
"""

import sys

sys.path.insert(0, "/opt/trn_rl_repo")

import ml_dtypes
import numpy as np

import concourse.bass as bass
import concourse.mybir as mybir
import concourse.tile as tile
from concourse import bacc
from concourse.bass_utils import run_bass_kernel_spmd
from concourse.masks import make_identity

N_NODES = 4096
IN_FEAT = 256
OUT_FEAT = 64
N_HEADS = 8
N_CORES = 8
ROWS = N_NODES // N_CORES          # 512 i-rows per core
F = N_HEADS * OUT_FEAT             # 512
JCH = N_NODES // 128               # 32 j-chunks
KCH = IN_FEAT // 128               # 2 k-chunks
E = OUT_FEAT + 1                   # 65: per-head lhsT cols (Wh | ones)

f32 = mybir.dt.float32
f32r = mybir.dt.float32r
bf16 = mybir.dt.bfloat16
AF = mybir.ActivationFunctionType
OP = mybir.AluOpType


def build_nc(exact=False, n_pool=22, lookahead=4, ts_ahead=2,
             pool_phase=0, g4_extra=6, n_pts=0, n_warm=4, sa_dve=True,
             pool_tail=3, pool_extra=(), fin0=4, fin_step=5, dma0=26,
             repeat=1):
    N_WARM = n_warm
    FIN0, FIN_STEP, DMA0 = fin0, fin_step, dma0
    """n_pool: how many of the 64 (head-group, j-chunk) passes run their
    adjacency mask-mult on GPSIMD instead of DVE (engine load balance).
    The last passes are kept on DVE so the 4.2us GPSIMD mask never sits
    on the end-of-kernel critical path."""
    mm_dt = f32r if exact else bf16
    wh_dt = f32 if exact else f32r
    nc = bacc.Bacc("TRN2", target_bir_lowering=False, debug=False,
                   num_devices=N_CORES)
    xT = nc.dram_tensor("xT", [IN_FEAT, N_NODES], wh_dt, kind="ExternalInput").ap()
    esrcD = nc.dram_tensor("esrc", [8, ROWS], wh_dt, kind="ExternalInput").ap()
    evqdD = nc.dram_tensor("evqd", [N_NODES // 8, 128], f32,
                           kind="ExternalInput").ap()
    Wd = nc.dram_tensor("W", [IN_FEAT, F], wh_dt, kind="ExternalInput").ap()
    selD = nc.dram_tensor("sel", [8, N_HEADS * 128], wh_dt,
                          kind="ExternalInput").ap()
    adjT = nc.dram_tensor("adjT", [N_NODES, ROWS], bf16, kind="ExternalInput").ap()
    out = nc.dram_tensor("out", [ROWS, F], f32, kind="ExternalOutput").ap()

    with tile.TileContext(nc) as tc:
      for rep in range(repeat):
        with tc.tile_pool(name=f"persist{rep}", bufs=1) as per, \
             tc.tile_pool(name=f"ph1sb{rep}", bufs=1) as ph1, \
             tc.tile_pool(name=f"work{rep}", bufs=1) as work, \
             tc.tile_pool(name=f"fin{rep}", bufs=1) as fin:
            # persistent SBUF (unique tags -> dedicated slots)
            wh1 = [per.tile([128, N_HEADS * E], mm_dt, tag=f"wh1_{jc}", name=f"wh1_{jc}")
                   for jc in range(JCH)]
            adjs = [per.tile([128, ROWS], bf16, tag=f"adj_{jc}", name=f"adj_{jc}")
                    for jc in range(JCH)]
            evqd_g = [per.tile([128, 128], f32, tag=f"evqd_g{g}",
                               name=f"evqd_g{g}") for g in range(4)]
            e8 = [per.tile([128, ROWS], mm_dt, tag=f"e8_{h}", name=f"e8_{h}")
                  for h in range(N_HEADS)]
            outsb = [per.tile([128, F], f32, tag=f"out_{k}", name=f"out_{k}")
                     for k in range(4)]
            espall = per.tile([8, ROWS], wh_dt, tag="espall", name="espall")
            # sel[k, h, p] = (k == h): one [8,128] slice per head is a
            # row-selector lhsT -- matmul(sel_h, espall) both selects row h
            # and broadcasts it across 128 partitions in one K=8 matmul.
            # (host-supplied so its dtype can be f32r for 1-cycle/row)
            sel = per.tile([8, N_HEADS * 128], wh_dt, tag="sel", name="sel")
            ones128 = per.tile([128, 8], f32, tag="ones128", name="ones128")
            ident = per.tile([128, 128], f32, tag="ident", name="ident")

            xt_sb = [ph1.tile([128, N_NODES], wh_dt, tag=f"xt{k}", name=f"xt{k}")
                     for k in range(KCH)]
            w_sb = [ph1.tile([128, F], wh_dt, tag=f"w{k}", name=f"w{k}")
                    for k in range(KCH)]

            nc.gpsimd.memset(ones128[:], 1.0)
            # dummy op pulls GPSIMD's one-time library-reload to t~1us
            # (otherwise it runs right before the first mask at ~9.5us)
            nc.gpsimd.tensor_tensor(ones128[:], ones128[:], ones128[:],
                                    OP.mult)
            make_identity(nc, ident[:])

            # --- input DMAs, all on the cheap sync queue. Gating chain
            # first (host-computed exp(0.8 e_src) + selector feed the E8
            # broadcast; host-computed exp(e_dst)/exp(0.2 e_dst) land per
            # group), then first adj/xT/W, then the rest interleaved.
            nc.sync.dma_start(espall[:], esrcD)
            nc.sync.dma_start(sel[:], selD)
            nc.sync.dma_start(evqd_g[0][:], evqdD[0:128, :])
            for jc in range(2):
                nc.sync.dma_start(adjs[jc][:],
                                  adjT[jc * 128:(jc + 1) * 128, :])
            for k in range(KCH):
                sl = slice(k * 128, (k + 1) * 128)
                nc.sync.dma_start(xt_sb[k][:][:, 0:512], xT[sl, 0:512])
            for k in range(KCH):
                sl = slice(k * 128, (k + 1) * 128)
                nc.sync.dma_start(w_sb[k][:], Wd[sl, :])
            for jc in range(2, 4):
                nc.sync.dma_start(adjs[jc][:],
                                  adjT[jc * 128:(jc + 1) * 128, :])
            for g in range(1, 4):
                nc.sync.dma_start(evqd_g[g][:],
                                  evqdD[g * 128:(g + 1) * 128, :])
            for c in range(1, 8):
                csl = slice(c * 512, (c + 1) * 512)
                for k in range(KCH):
                    sl = slice(k * 128, (k + 1) * 128)
                    nc.sync.dma_start(xt_sb[k][:][:, csl], xT[sl, csl])
                for jc in range(4 * c, 4 * c + 4):
                    nc.sync.dma_start(adjs[jc][:],
                                      adjT[jc * 128:(jc + 1) * 128, :])

            # --- main: two sweeps over j-chunks (head groups 0-3, 4-7),
            # Wh/e-vector staging software-pipelined into sweep 1 ---
            passes = [(tuple(range(hg * 4, hg * 4 + 4)), jc)
                      for hg in range(2) for jc in range(JCH)]
            # split the final pass into head pairs so heads 4-5 finalize
            # while 6-7 are still accumulating their last chunk
            passes[-1:] = [((4, 5), JCH - 1), ((6, 7), JCH - 1)]
            NP = len(passes)

            NPP = 64 - pool_tail             # no pool masks on the tail

            def pool_pass(p):
                if p in pool_extra:
                    return True
                q = p + pool_phase
                return (p < NPP
                        and ((q + 1) * n_pool) // NPP > (q * n_pool) // NPP)

            # idx-0 tensor_scalar of every k-th pool pass also on GPSIMD
            # (feeds Pool's own mask -- no cross-engine hop back to DVE)
            _pp = [p for p in range(NP) if pool_pass(p)]
            pool_ts_set = set(_pp[1::max(1, len(_pp) // n_pts)][:n_pts]
                              if n_pts else [])


            g4_of = {}

            def emit_ts(p):
                heads, jc = passes[p]
                g, o = jc // 8, jc % 8
                nh = len(heads)
                g4 = work.tile([128, nh * ROWS], mm_dt, tag=f"g{nh}",
                               name=f"g{nh}", bufs=ts_ahead + g4_extra)
                g4_of[p] = g4
                for idx, h in enumerate(heads):
                    qap = evqd_g[g][:][:, o * 8 + h:o * 8 + h + 1]
                    dap = evqd_g[g][:][:, 64 + o * 8 + h:64 + o * 8 + h + 1]
                    eng = (nc.gpsimd if (idx == 0 and p in pool_ts_set)
                           else nc.vector)
                    eng.tensor_scalar(
                        g4[:][:, idx * ROWS:(idx + 1) * ROWS],
                        e8[h][:], dap, qap, OP.mult, OP.max)

            def emit_mask_mms(p, acc):
                heads, jc = passes[p]
                nh = len(heads)
                g4 = g4_of.pop(p)
                eng = nc.gpsimd if pool_pass(p) else nc.vector
                eng.tensor_tensor(
                    g4[:].rearrange("p (b r) -> p b r", b=nh),
                    g4[:].rearrange("p (b r) -> p b r", b=nh),
                    adjs[jc][:].unsqueeze(1).broadcast_to([128, nh, ROWS]),
                    OP.mult)
                for idx, h in enumerate(heads):
                    nc.tensor.matmul(
                        acc[h][:],
                        wh1[jc][:][:, h * E:(h + 1) * E],
                        g4[:][:, idx * ROWS:(idx + 1) * ROWS],
                        start=(jc == 0), stop=(jc == JCH - 1))

            def finalize_head(h, tpp, relu_dve=False):
                tp = tpp.tile([128, 4 * E], f32, tag="tp", name="tp")
                rec = fin.tile([128, 4], f32, tag="rec", name="rec", bufs=2)
                for k in range(4):
                    ksl = slice(k * 128, (k + 1) * 128)
                    nc.tensor.transpose(
                        tp[:][:, k * E:(k + 1) * E],
                        sA[h][:][:, ksl], ident[0:E, 0:E])
                nc.vector.reciprocal(
                    rec[:],
                    tp[:].rearrange("p (k e) -> p k e", k=4)[:, :, OUT_FEAT:E])
                for k in range(4):
                    if relu_dve:
                        nc.vector.tensor_scalar(
                            outsb[k][:][:, h * OUT_FEAT:(h + 1) * OUT_FEAT],
                            tp[:][:, k * E:k * E + OUT_FEAT],
                            rec[:][:, k:k + 1], 0.0, OP.mult, OP.max)
                    else:
                        nc.scalar.activation(
                            outsb[k][:][:, h * OUT_FEAT:(h + 1) * OUT_FEAT],
                            tp[:][:, k * E:k * E + OUT_FEAT],
                            AF.Relu, scale=rec[:][:, k:k + 1])

            sA = {}
            with tc.tile_pool(name=f"stgps{rep}", bufs=1,
                              space="PSUM") as stgps:

                def stage(jc):
                    # Wh staging: matmuls + bf16 [Wh_h | 1] lhsT blocks
                    # (Wh cols on ACT -- DVE is the bottleneck engine)
                    jsl = slice(jc * 128, (jc + 1) * 128)
                    whp = stgps.tile([128, F], f32, tag="whp", name="whp",
                                     bufs=3)
                    for k in range(KCH):
                        nc.tensor.matmul(
                            whp[:], xt_sb[k][:][:, jsl], w_sb[k][:],
                            start=(k == 0), stop=(k == KCH - 1))
                    dst = wh1[jc][:].rearrange("p (h e) -> p h e", h=N_HEADS)
                    nc.scalar.copy(
                        dst[:, :, 0:OUT_FEAT],
                        whp[:].rearrange("p (h d) -> p h d", h=N_HEADS))
                    nc.scalar.copy(
                        dst[:, :, OUT_FEAT:E],
                        ones128[:].rearrange("p (e o) -> p e o", o=1))

                # prologue: e_src -> E8 broadcast, interleaved with the
                # first staged chunks so the first pass's e-vectors don't
                # queue on ACT behind all eight E8 activations. Dummy
                # transposes pre-warm the Tensor engine to full p-state
                # while the input DMAs land.
                with tc.tile_pool(name=f"props{rep}", bufs=1,
                                  space="PSUM") as props:
                    for w in range(N_WARM):
                        warm = props.tile([128, 128], f32, tag="warm",
                                          name="warm", bufs=1)
                        nc.tensor.transpose(warm[:], ident[:], ident[:])

                    def bp_e8(h):
                        bp = props.tile([128, ROWS], f32, tag="bp",
                                        name="bp", bufs=4)
                        nc.tensor.matmul(bp[:],
                                         sel[:][:, h * 128:(h + 1) * 128],
                                         espall[:],
                                         start=True, stop=True)
                        nc.scalar.copy(e8[h][:], bp[:])

                    for h in range(4):
                        bp_e8(h)
                    for jc in range(lookahead):
                        stage(jc)
                    for h in range(4, 8):
                        bp_e8(h)

                with tc.tile_pool(name=f"acc{rep}_0", bufs=1,
                                  space="PSUM") as accp:
                    acc = {h: accp.tile([E, ROWS], f32, tag=f"acc{h}",
                                        name=f"acc{h}")
                           for h in range(4)}
                    for p in range(ts_ahead):
                        emit_ts(p)
                    for p in range(JCH):
                        jc = passes[p][1]
                        if jc + lookahead < JCH:
                            stage(jc + lookahead)
                        if p + ts_ahead < NP:
                            emit_ts(p + ts_ahead)
                        emit_mask_mms(p, acc)
                    for h in range(4):
                        sA[h] = fin.tile([E, ROWS], f32, tag=f"sA{h % 4}",
                                         name=f"sA{h}", bufs=2)
                        nc.scalar.copy(sA[h][:], acc[h][:])

            # sweep 2 (heads 4-7); head-group-0 finalize interleaved
            fin_sched = {JCH + FIN0 + FIN_STEP * h: h for h in range(4)}
            with tc.tile_pool(name=f"acc{rep}_1", bufs=1,
                              space="PSUM") as accp, \
                 tc.tile_pool(name=f"tpp{rep}", bufs=4,
                              space="PSUM") as tpp:
                acc = {h: accp.tile([E, ROWS], f32, tag=f"acc{h}",
                                    name=f"acc{h}")
                       for h in range(4, 8)}
                dve_copy = lambda o, i: nc.vector.tensor_copy(o, i)
                sA_eng = {4: dve_copy if sa_dve else nc.scalar.copy,
                          5: nc.scalar.copy,
                          6: dve_copy if sa_dve else nc.scalar.copy,
                          7: nc.scalar.copy}

                def tail_pair(pair):
                    # PSUM->SBUF staging and the relu-scale both split
                    # across ACT/DVE so the pair's chains run in parallel
                    for h in pair:
                        sA[h] = fin.tile([E, ROWS], f32, tag=f"sA{h % 4}",
                                         name=f"sA{h}", bufs=2)
                        sA_eng[h](sA[h][:], acc[h][:])
                    for h in pair:
                        finalize_head(h, tpp, relu_dve=(h % 2 == 1))

                for p in range(JCH, NP):
                    if p + ts_ahead < NP:
                        emit_ts(p + ts_ahead)
                    emit_mask_mms(p, acc)
                    if p in fin_sched:
                        finalize_head(fin_sched[p], tpp)
                    if p == JCH + DMA0:
                        for k in range(4):
                            nc.sync.dma_start(
                                out[k * 128:(k + 1) * 128, 0:4 * OUT_FEAT],
                                outsb[k][:][:, 0:4 * OUT_FEAT])
                    if p == NP - 2:
                        tail_pair((4, 5))
                tail_pair((6, 7))
                for k in range(4):
                    nc.sync.dma_start(
                        out[k * 128:(k + 1) * 128, 4 * OUT_FEAT:F],
                        outsb[k][:][:, 4 * OUT_FEAT:F])

    nc.compile()
    return nc


_NC_CACHE = {}


def get_nc(exact=False, **kw):
    key = (exact, tuple(sorted(kw.items())))
    if key not in _NC_CACHE:
        _NC_CACHE[key] = build_nc(exact, **kw)
    return _NC_CACHE[key]


def make_in_maps(x, adj, W, a):
    x = np.asarray(x, dtype=np.float32)
    adj = np.asarray(adj, dtype=np.float32)
    W = np.asarray(W, dtype=np.float32)
    a = np.asarray(a, dtype=np.float32)

    xT = np.ascontiguousarray(x.T)                       # [256, 4096]
    a_src = a[:, :OUT_FEAT].astype(np.float64)           # [8, 64]
    a_dst = a[:, OUT_FEAT:].astype(np.float64)
    W3 = W.astype(np.float64).reshape(IN_FEAT, N_HEADS, OUT_FEAT)
    wa_dst = np.einsum("khd,hd->kh", W3, a_dst)          # [256, 8]
    wa_src = np.einsum("khd,hd->kh", W3, a_src)

    x64 = x.astype(np.float64)
    e_dst = x64 @ wa_dst                                  # [4096, 8]
    # evqd[g*128+p, o*8+h] = exp(0.2 e_dst[j,h]); cols 64+: exp(e_dst)
    # with j = (g*8+o)*128 + p  (group/chunk/partition tile layout)
    ed = e_dst.reshape(4, 8, 128, N_HEADS).transpose(0, 2, 1, 3)
    evqd = np.stack(
        [np.exp(0.2 * ed), np.exp(ed)], axis=2).reshape(512, 128)
    sel = np.zeros((8, N_HEADS * 128), dtype=np.float32)
    for h in range(N_HEADS):
        sel[h, h * 128:(h + 1) * 128] = 1.0

    in_maps = []
    for c in range(N_CORES):
        rs = slice(c * ROWS, (c + 1) * ROWS)
        in_maps.append({
            "xT": xT,
            "esrc": np.ascontiguousarray(
                np.exp(0.8 * (x64[rs] @ wa_src)).T.astype(np.float32)),
            "evqd": evqd.astype(np.float32),
            "W": W,
            "sel": sel,
            "adjT": np.ascontiguousarray(adj[rs, :].T).astype(ml_dtypes.bfloat16),
        })
    return in_maps


def kernel(x, adj, W, a):
    import os
    exact = os.environ.get("GAT_EXACT", "0") == "1"
    # default: bf16 attention weights + lhsT, ~3e-3 max rel err.
    # GAT_EXACT=1: f32/f32r matmuls, ~1.5e-3, slower.
    nc = get_nc(exact=exact)
    in_maps = make_in_maps(x, adj, W, a)
    res = run_bass_kernel_spmd(nc, in_maps, core_ids=list(range(N_CORES)))
    return np.concatenate([res.results[c]["out"] for c in range(N_CORES)],
                          axis=0)


if __name__ == "__main__":
    rng = np.random.default_rng(0)
    x = rng.standard_normal((N_NODES, IN_FEAT), dtype=np.float32)
    adj = (rng.random((N_NODES, N_NODES)) < 0.01).astype(np.float32)
    np.fill_diagonal(adj, 1.0)
    W = (rng.standard_normal((256, F), dtype=np.float32) * 0.05)
    a = rng.standard_normal((N_HEADS, 2 * OUT_FEAT), dtype=np.float32)
    out = kernel(x=x, adj=adj, W=W, a=a)
    print("out", out.shape, out.dtype, float(np.abs(out).max()))
